# revision 2
# baseline (speedup 1.0000x reference)
"""Trainium2 Bass kernel for nn_DecoderLSTMCell (ragged packed-sequence LSTM
decoder + vocab projection).

Strategy: data-parallel over the batch across 8 NeuronCores with *strided*
row assignment (rows sorted by length descending, so core k takes rows
k, k+8, ... and every core sees a near-identical shrinking active set).
Everything on-device runs in a transposed layout ([feature-on-partitions,
token-on-free]) so per-(4H-row) biases are ACT per-partition scalars and the
recurrent state feeds matmuls directly.  Matmul operands are bf16 with fp32
PSUM accumulation; state (h, c) and activations are fp32.

Because lengths are sorted descending, the active batch prefix shrinks
monotonically: step t only updates the first a_t rows, and the packed
(h, c) history *is* the state (step t+1 reads the first a_{t+1} columns of
step t's output).  No masking is ever needed.

Per core:
  vemb.T = visual_W @ features.T + visual_b            (t=0 token columns)
  X.T    = [vemb.T | embed gather (host)]              packed [E, ptok]
  gx.T   = W_ih @ X.T + (b_ih + b_hh)                  packed [4H, ptok]
  step t: gates.T = gx.T[:, t] + W_hh @ hbf[:, t-1];   i,f,g,o -> c, h
  logits.T = out_W @ hbf + out_b                       [V, ptok]

Host does input prep only (transposes/casts/embedding gather = indexing) and
reassembles the packed outputs into the reference's global packed order.
"""

import math
import os
from contextlib import ExitStack

import numpy as np
import ml_dtypes

import concourse.bass as bass
import concourse.bacc as bacc
import concourse.tile as tile
import concourse.mybir as mybir
from concourse.bass_utils import run_bass_kernel_spmd

AF = mybir.ActivationFunctionType
FP32 = mybir.dt.float32
BF16 = mybir.dt.bfloat16
bf16 = ml_dtypes.bfloat16

N_CORES = 8
TOKC = 512  # token-chunk width for big matmuls (<= one PSUM bank of fp32)

LAST_PERF = {}
LAST_RUN = {}


# ----------------------------------------------------------------------------
# schedule
# ----------------------------------------------------------------------------

def _schedule(lens, n_cores):
    """Per-step padded active counts p_t = ceil(bs_t / n_cores), offsets."""
    lens = np.asarray(lens).astype(np.int64)
    T = int(lens.max())
    bs = (lens[None, :] > np.arange(T)[:, None]).sum(1).astype(np.int64)  # [T]
    p = np.ceil(bs / n_cores).astype(np.int64)
    off = np.concatenate([[0], np.cumsum(p)]).astype(np.int64)
    return T, bs, p, off


# ----------------------------------------------------------------------------
# device program (one SPMD program; cores differ only in input data)
# ----------------------------------------------------------------------------

def _build_program(nb, E, F, H, VC, ptok, p, off, amax):
    nE, nF, nH, nM = E // 128, F // 128, H // 128, 4 * H // 128
    T_steps = len(p)
    n_tokc = math.ceil(ptok / TOKC)

    nc = bacc.Bacc("TRN2", target_bir_lowering=False, debug=False,
                   num_devices=N_CORES)

    xt_d = nc.dram_tensor("xt", [128, nE, ptok], BF16, kind="ExternalInput")
    whh_d = nc.dram_tensor("whh", [128, nH, nM, 128], BF16, kind="ExternalInput")
    wih_d = nc.dram_tensor("wih", [128, nE, nM, 128], BF16, kind="ExternalInput")
    outw_d = nc.dram_tensor("outw", [VC, 128, nH, 128], BF16, kind="ExternalInput")
    vw_d = nc.dram_tensor("vw", [128, nF, nE, 128], BF16, kind="ExternalInput")
    ft_d = nc.dram_tensor("ft", [128, nF, nb], BF16, kind="ExternalInput")
    b4_d = nc.dram_tensor("b4", [128, nM], FP32, kind="ExternalInput")
    ob_d = nc.dram_tensor("ob", [128, VC], FP32, kind="ExternalInput")
    vb_d = nc.dram_tensor("vb", [128, nE], FP32, kind="ExternalInput")

    lt_d = nc.dram_tensor("logitsT", [VC, 128, ptok], FP32, kind="ExternalOutput")
    ht_d = nc.dram_tensor("hT", [nH, 128, ptok], FP32, kind="ExternalOutput")
    ct_d = nc.dram_tensor("cT", [nH, 128, ptok], FP32, kind="ExternalOutput")

    with tile.TileContext(nc) as tc, ExitStack() as ctx:
        persist = ctx.enter_context(tc.tile_pool(name="persist", bufs=1))
        wpool = ctx.enter_context(tc.tile_pool(name="wstream", bufs=3))
        ewpool = ctx.enter_context(tc.tile_pool(name="ew", bufs=2))
        winpool = ctx.enter_context(tc.tile_pool(name="win", bufs=2))
        lbpool = ctx.enter_context(tc.tile_pool(name="lb", bufs=4))
        ps_g = ctx.enter_context(tc.tile_pool(name="psg", bufs=1, space="PSUM"))
        ps_x = ctx.enter_context(tc.tile_pool(name="psx", bufs=2, space="PSUM"))
        ps_l = ctx.enter_context(tc.tile_pool(name="psl", bufs=2, space="PSUM"))

        XT = persist.tile([128, nE, ptok], BF16)
        GX = persist.tile([128, nM, ptok], BF16)
        HBF = persist.tile([128, nH, ptok], BF16)
        WHH = persist.tile([128, nH, nM, 128], BF16)
        B4 = persist.tile([128, nM], FP32)
        OB = persist.tile([128, VC], FP32)
        VB = persist.tile([128, nE], FP32)
        FT = persist.tile([128, nF, nb], BF16)

        nc.sync.dma_start(XT[:], xt_d[:])
        nc.sync.dma_start(WHH[:], whh_d[:])
        nc.sync.dma_start(B4[:], b4_d[:])
        nc.sync.dma_start(OB[:], ob_d[:])
        nc.sync.dma_start(VB[:], vb_d[:])
        nc.sync.dma_start(FT[:], ft_d[:])

        # ---- vemb.T -> XT[:, :, 0:nb] (t = 0 token columns) ----
        for me in range(nE):
            vwt = wpool.tile([128, nF, 128], BF16, tag="w")
            nc.sync.dma_start(vwt[:], vw_d[:, :, me, :])
            pv = ps_x.tile([128, TOKC], FP32, tag="px")
            for kf in range(nF):
                nc.tensor.matmul(pv[:, :nb], vwt[:, kf, :], FT[:, kf, :],
                                 start=(kf == 0), stop=(kf == nF - 1))
            nc.scalar.activation(XT[:, me, 0:nb], pv[:, :nb], AF.Identity,
                                 bias=VB[:, me:me + 1])

        # ---- xproj chunks (emitted lazily, interleaved with recurrence) ----
        emitted = [0]  # token columns of GX produced so far

        def emit_xproj_chunk():
            c0 = emitted[0]
            w = min(TOKC, ptok - c0)
            for mc in range(nM):
                wt = wpool.tile([128, nE, 128], BF16, tag="w")
                nc.sync.dma_start(wt[:], wih_d[:, :, mc, :])
                px = ps_x.tile([128, TOKC], FP32, tag="px")
                for ke in range(nE):
                    nc.tensor.matmul(px[:, :w], wt[:, ke, :], XT[:, ke, c0:c0 + w],
                                     start=(ke == 0), stop=(ke == nE - 1))
                nc.scalar.activation(GX[:, mc, c0:c0 + w], px[:, :w], AF.Identity,
                                     bias=B4[:, mc:mc + 1])
            emitted[0] = c0 + w

        # ---- recurrence ----
        c_prev = None
        for t in range(T_steps):
            a = int(p[t])
            o0 = int(off[t])
            op0 = int(off[t - 1]) if t > 0 else 0
            while emitted[0] < o0 + a:
                emit_xproj_chunk()

            gates = []
            for g in range(4):
                gsb = ewpool.tile([128, nH, amax], FP32, tag=f"gs{g}")
                if t > 0:
                    psg = ps_g.tile([128, nH, amax], FP32, tag=f"g{g}")
                    for sub in range(nH):
                        mc = g * nH + sub
                        for kh in range(nH):
                            nc.tensor.matmul(
                                psg[:, sub, :a], WHH[:, kh, mc, :],
                                HBF[:, kh, op0:op0 + a],
                                start=(kh == 0), stop=(kh == nH - 1))
                    nc.vector.tensor_add(gsb[:, :, :a], psg[:, :, :a],
                                         GX[:, g * nH:(g + 1) * nH, o0:o0 + a])
                else:
                    nc.vector.tensor_copy(gsb[:, :, :a],
                                          GX[:, g * nH:(g + 1) * nH, o0:o0 + a])
                gates.append(gsb)

            acts = []
            for gi, fn in ((0, AF.Sigmoid), (1, AF.Sigmoid), (2, AF.Tanh),
                           (3, AF.Sigmoid)):
                asb = ewpool.tile([128, nH, amax], FP32, tag=f"a{gi}")
                nc.scalar.activation(asb[:, :, :a], gates[gi][:, :, :a], fn)
                acts.append(asb)
            i_s, f_s, g_s, o_s = acts

            c_cur = winpool.tile([128, nH, amax], FP32, tag="c")
            if t > 0:
                tmp = ewpool.tile([128, nH, amax], FP32, tag="tmp")
                fc = ewpool.tile([128, nH, amax], FP32, tag="fc")
                nc.vector.tensor_mul(tmp[:, :, :a], i_s[:, :, :a], g_s[:, :, :a])
                nc.vector.tensor_mul(fc[:, :, :a], f_s[:, :, :a],
                                     c_prev[:, :, :a])
                nc.vector.tensor_add(c_cur[:, :, :a], fc[:, :, :a],
                                     tmp[:, :, :a])
            else:
                nc.vector.tensor_mul(c_cur[:, :, :a], i_s[:, :, :a],
                                     g_s[:, :, :a])

            ct = ewpool.tile([128, nH, amax], FP32, tag="ct")
            nc.scalar.activation(ct[:, :, :a], c_cur[:, :, :a], AF.Tanh)
            h_cur = winpool.tile([128, nH, amax], FP32, tag="h")
            nc.vector.tensor_mul(h_cur[:, :, :a], o_s[:, :, :a], ct[:, :, :a])

            nc.vector.tensor_copy(HBF[:, :, o0:o0 + a], h_cur[:, :, :a])
            for pl in range(nH):
                nc.sync.dma_start(ht_d[pl, :, o0:o0 + a], h_cur[:, pl, :a])
                nc.sync.dma_start(ct_d[pl, :, o0:o0 + a], c_cur[:, pl, :a])
            c_prev = c_cur

        while emitted[0] < ptok:  # any tail tokens (shouldn't happen)
            emit_xproj_chunk()

        # ---- logits.T ----
        for vc in range(VC):
            wt = wpool.tile([128, nH, 128], BF16, tag="w")
            nc.sync.dma_start(wt[:], outw_d[vc])
            for tcx in range(n_tokc):
                c0 = tcx * TOKC
                w = min(TOKC, ptok - c0)
                pl_ = ps_l.tile([128, TOKC], FP32, tag="pl")
                for kh in range(nH):
                    nc.tensor.matmul(pl_[:, :w], wt[:, kh, :],
                                     HBF[:, kh, c0:c0 + w],
                                     start=(kh == 0), stop=(kh == nH - 1))
                lb = lbpool.tile([128, TOKC], FP32, tag="lb")
                nc.scalar.activation(lb[:, :w], pl_[:, :w], AF.Identity,
                                     bias=OB[:, vc:vc + 1])
                nc.sync.dma_start(lt_d[vc, :, c0:c0 + w], lb[:, :w])

    nc.compile()
    return nc


# ----------------------------------------------------------------------------
# host-side packing
# ----------------------------------------------------------------------------

def _pack_core_inputs(k, ins, dims, T_steps, p, off, ptok):
    nb, E, F, H, VC = dims["nb"], dims["E"], dims["F"], dims["H"], dims["VC"]
    nE, nF, nH, nM = E // 128, F // 128, H // 128, 4 * H // 128
    B = ins["features"].shape[0]
    rows = np.arange(k, B, N_CORES)

    caps = ins["target_caps"]
    Emb = ins["embed_table"]

    # X.T packed [E, ptok] -> dram layout [128, nE, ptok]
    XT = np.zeros((E, ptok), np.float32)
    col_rows, col_ts, col_pos = [], [], []
    for t in range(1, T_steps):
        a = int(p[t])
        col_rows.append(rows[:a])
        col_ts.append(np.full(a, t - 1))
        col_pos.append(np.arange(off[t], off[t] + a))
    if col_rows:
        col_rows = np.concatenate(col_rows)
        col_ts = np.concatenate(col_ts)
        col_pos = np.concatenate(col_pos)
        XT[:, col_pos] = Emb[caps[col_rows, col_ts]].T
    xt = np.ascontiguousarray(
        XT.reshape(nE, 128, ptok).transpose(1, 0, 2)).astype(bf16)

    ft = np.ascontiguousarray(
        ins["features"][rows].reshape(nb, nF, 128).transpose(2, 1, 0)
    ).astype(bf16)
    return {"xt": xt, "ft": ft}


def _pack_shared_inputs(ins, dims):
    E, F, H, VC, V = dims["E"], dims["F"], dims["H"], dims["VC"], dims["V"]
    nE, nF, nH, nM = E // 128, F // 128, H // 128, 4 * H // 128

    W_hh, W_ih = ins["W_hh"], ins["W_ih"]
    whh = np.ascontiguousarray(
        W_hh.reshape(nM, 128, nH, 128).transpose(3, 2, 0, 1)).astype(bf16)
    wih = np.ascontiguousarray(
        W_ih.reshape(nM, 128, nE, 128).transpose(3, 2, 0, 1)).astype(bf16)

    outW = ins["out_W"]
    outWp = np.zeros((VC * 128, H), np.float32)
    outWp[:V] = outW
    outw = np.ascontiguousarray(
        outWp.reshape(VC, 128, nH, 128).transpose(0, 3, 2, 1)).astype(bf16)

    vw = np.ascontiguousarray(
        ins["visual_W"].reshape(nE, 128, nF, 128).transpose(3, 2, 0, 1)
    ).astype(bf16)

    b4 = np.ascontiguousarray(
        (ins["b_ih"] + ins["b_hh"]).reshape(nM, 128).T).astype(np.float32)
    obp = np.zeros(VC * 128, np.float32)
    obp[:V] = ins["out_b"]
    ob = np.ascontiguousarray(obp.reshape(VC, 128).T).astype(np.float32)
    vb = np.ascontiguousarray(
        ins["visual_b"].reshape(nE, 128).T).astype(np.float32)
    return {"whh": whh, "wih": wih, "outw": outw, "vw": vw,
            "b4": b4, "ob": ob, "vb": vb}


# ----------------------------------------------------------------------------
# entry point
# ----------------------------------------------------------------------------

def kernel(**inputs):
    ins = {k: np.asarray(v) for k, v in inputs.items()}
    for k in ("features", "visual_W", "visual_b", "embed_table", "W_ih",
              "W_hh", "b_ih", "b_hh", "out_W", "out_b"):
        ins[k] = ins[k].astype(np.float32)
    lens = np.asarray(ins["lengths"]).astype(np.int64)
    caps_dtype = inputs["target_caps"].dtype

    B, F = ins["features"].shape
    V, H = ins["out_W"].shape
    E = ins["embed_table"].shape[1]
    VC = math.ceil(V / 128)
    nb = B // N_CORES
    assert B % N_CORES == 0

    T_steps, bs, p, off = _schedule(lens, N_CORES)
    ptok = int(off[-1])
    amax = int(p.max())
    dims = dict(nb=nb, E=E, F=F, H=H, VC=VC, V=V)

    nc = _build_program(nb, E, F, H, VC, ptok, p, off, amax)

    shared = _pack_shared_inputs(ins, dims)
    in_maps = []
    for k in range(N_CORES):
        m = dict(shared)
        m.update(_pack_core_inputs(k, ins, dims, T_steps, p, off, ptok))
        in_maps.append(m)

    res = run_bass_kernel_spmd(nc, in_maps, core_ids=list(range(N_CORES)))
    LAST_PERF.clear()
    LAST_PERF.update(exec_time_ns=res.exec_time_ns)
    LAST_RUN.clear()
    LAST_RUN.update(nc=nc, in_maps=in_maps)

    # ---- reassemble ----
    total = int(bs.sum())
    goff = np.concatenate([[0], np.cumsum(bs)])
    logits = np.empty((total, V), np.float32)
    h_states = np.empty((total, H), np.float32)
    c_states = np.empty((total, H), np.float32)

    for k in range(N_CORES):
        out = res.results[k]
        lt = out["logitsT"].reshape(VC * 128, ptok)
        ht = out["hT"].reshape(H, ptok)
        ct = out["cT"].reshape(H, ptok)
        rows = np.arange(k, B, N_CORES)
        a_k = (lens[rows][None, :] > np.arange(T_steps)[:, None]).sum(1)
        gpos, lpos = [], []
        for t in range(T_steps):
            ak = int(a_k[t])
            if ak == 0:
                continue
            gpos.append(goff[t] + rows[:ak])
            lpos.append(off[t] + np.arange(ak))
        gpos = np.concatenate(gpos)
        lpos = np.concatenate(lpos)
        logits[gpos] = lt[:V, lpos].T
        h_states[gpos] = ht[:, lpos].T
        c_states[gpos] = ct[:, lpos].T

    t_last = int(lens[0]) - 1
    bs_last = int(bs[t_last])
    h_fin = h_states[goff[t_last]:goff[t_last] + bs_last].copy()
    c_fin = c_states[goff[t_last]:goff[t_last] + bs_last].copy()
    del caps_dtype
    return logits, (h_fin, c_fin), (h_states, c_states)
